# revision 6
# baseline (speedup 1.0000x reference)
"""Trainium2 kernel for ApplyStickerLayer: out = roll(subimg, (80,80), (2,3)) + base_image.

Structure (guaranteed by the layer): subimg is zero outside the 50x50 sticker
at the origin, base_image is zero inside the destination window, and the roll
never wraps -- so per (b, c) channel image (flat, 50176 elems):

    out[b,c] = base[c] + shift_by_18000(sub[b,c])

Only rows 80..129 (MID, flat [17920, 29120) per channel) depend on the batch.
Everything else is a broadcast of base.  Key layout fact: BOT of channel image
bc is contiguous with TOP of image bc+1, so the non-MID output is

    head TOP[0] | 95 x segment(bc) | tail BOT[95]

where segment(bc) = BOT[bc%3] ++ TOP[(bc+1)%3] is a 38976-elem (155.9 KB)
contiguous run with only THREE distinct contents.  HW measurement: an SDMA
descriptor costs ~700 ns regardless of size (HBM write round-trip), so the
whole game is maximal descriptors.  Segments give 155.9 KB descriptors at
engine line rate (~27 GiB/s); R replicas per variant in SBUF let one store
cover R segments (SWDGE can't do stride-0 sources).

MID is composed on-chip with one TensorE pass per column chunk:

    psum[96, f] = W.T @ x      W [99, 96] = [identity ; channel selector]
                               x [99, f]  = [96 sub rows ; 3 base mid rows]

The sub rows are DMA'd into x with a +80 column offset, which lands the
sticker at columns 80..129 of each row (the spill-over is all zeros by the
sparsity guarantee).  PSUM chunks are copied to SBUF by DVE and stored as
96 fat descriptors per pass.

Per core: ~19.3 MB written + ~5 MB read => HBM roofline ~68 us.
"""

import sys

import numpy as np

if "/opt/trn_rl_repo" not in sys.path:
    sys.path.insert(0, "/opt/trn_rl_repo")

import concourse.bacc as bacc
import concourse.bass as bass
import concourse.mybir as mybir
import concourse.tile as tile
from concourse.bass_utils import run_bass_kernel_spmd

N_CORES = 8
B, C, H, W = 256, 3, 224, 224
BS = B // N_CORES  # 32 batches per core
BC = BS * C  # 96 channel images per core
SH, SW = 80, 80
KH, KW = 50, 50

CHW = H * W  # 50176
IMG = C * CHW  # 150528

TOP_LEN = SH * W  # 17920
MID_OFF, MID_LEN = TOP_LEN, KH * W  # [17920, 29120)
BOT_OFF = MID_OFF + MID_LEN  # 29120
BOT_LEN = CHW - BOT_OFF  # 21056
SEG_LEN = BOT_LEN + TOP_LEN  # 38976: BOT[bc] ++ TOP[bc+1]
N_SEG = BC - 1  # 95 interior segments

_F32 = mybir.dt.float32

DEFAULT_CFG = {
    "rep": 4,  # segment replicas per variant in SBUF
    "n_pass": 2,  # MID column passes (x load granularity)
    "mm_f": 512,  # matmul free-dim chunk (<= 512, one PSUM bank)
    "psum_bufs": 4,
    "swq": 1,  # num_swdge_queues
}


def build_nc(cfg=None):
    cfg = {**DEFAULT_CFG, **(cfg or {})}
    rep = cfg["rep"]
    n_pass = cfg["n_pass"]
    pass_len = MID_LEN // n_pass
    assert pass_len * n_pass == MID_LEN
    mm_f = cfg["mm_f"]
    K = BC + C  # 99: matmul contraction (96 sub rows + 3 base rows)

    nc = bacc.Bacc(
        "TRN2",
        target_bir_lowering=False,
        num_devices=N_CORES,
        num_swdge_queues=cfg["swq"],
    )
    sub = nc.declare_dram_parameter("subimg", [BS, C, H, W], _F32, isOutput=False)
    base = nc.declare_dram_parameter("base", [C, H, W], _F32, isOutput=False)
    wsel = nc.declare_dram_parameter("wsel", [K, BC], _F32, isOutput=False)
    out = nc.declare_dram_parameter("out", [BS, C, H, W], _F32, isOutput=True)

    with tile.TileContext(nc) as tc:
        with (
            tc.tile_pool(name="consts", bufs=1) as cpool,
            tc.tile_pool(name="psum", bufs=cfg["psum_bufs"], space=bass.MemorySpace.PSUM) as ppool,
        ):
            # ---- segment tile: rows v*rep + r = replica r of variant v ----
            # variant v = BOT[channel v] ++ TOP[channel (v+1)%3]
            t_seg = cpool.tile([3 * rep, SEG_LEN], _F32, tag="seg")
            for v in range(3):
                nc.gpsimd.dma_start(
                    out=t_seg[v * rep : v * rep + 1, 0:BOT_LEN],
                    in_=bass.AP(base, v * CHW + BOT_OFF, [[BOT_LEN, 1], [1, BOT_LEN]]),
                )
                nc.gpsimd.dma_start(
                    out=t_seg[v * rep : v * rep + 1, BOT_LEN:SEG_LEN],
                    in_=bass.AP(base, ((v + 1) % 3) * CHW, [[TOP_LEN, 1], [1, TOP_LEN]]),
                )
                k = 1
                while k < rep:  # replicate within the variant's row block
                    k2 = min(2 * k, rep)
                    nc.gpsimd.dma_start(
                        out=t_seg[v * rep + k : v * rep + k2, :],
                        in_=t_seg[v * rep : v * rep + (k2 - k), :],
                    )
                    k = k2

            # ---- matmul weights ----
            t_w = cpool.tile([K, BC], _F32, tag="w")
            nc.gpsimd.dma_start(out=t_w[:, :], in_=wsel[:, :])

            # ---- head / tail (pieces of variant 2: BOT[2] ++ TOP[0]) ----
            nc.gpsimd.dma_start(
                out=bass.AP(out, 0, [[TOP_LEN, 1], [1, TOP_LEN]]),
                in_=t_seg[2 * rep : 2 * rep + 1, BOT_LEN:SEG_LEN],
            )
            nc.gpsimd.dma_start(
                out=bass.AP(out, (BC - 1) * CHW + BOT_OFF, [[BOT_LEN, 1], [1, BOT_LEN]]),
                in_=t_seg[2 * rep : 2 * rep + 1, 0:BOT_LEN],
            )

            # ---- interior segment stores: segment j covers
            #      out flat [j*CHW + BOT_OFF, (j+1)*CHW + TOP_LEN), j = 0..94 ----
            for v in range(3):
                segs = list(range(v, N_SEG, 3))
                for i in range(0, len(segs), rep):
                    g = min(rep, len(segs) - i)
                    j0 = segs[i]
                    nc.gpsimd.dma_start(
                        out=bass.AP(
                            out, j0 * CHW + BOT_OFF, [[IMG, g], [1, SEG_LEN]]
                        ),
                        in_=t_seg[v * rep : v * rep + g, :],
                    )

            # ---- MID: per pass, load base+sub into x, matmul, evac, store ----
            t_x = cpool.tile([K, SW + pass_len], _F32, tag="x")
            for ps in range(n_pass):
                c0 = ps * pass_len  # column offset inside the MID band
                t_b = cpool.tile([BC, pass_len], _F32, tag="bout")
                nc.gpsimd.dma_start(
                    out=t_x[BC:K, 0:pass_len],
                    in_=bass.AP(base, MID_OFF + c0, [[CHW, C], [1, pass_len]]),
                )
                if ps == 0:
                    # x[bc, j] = sub[bc, j - 80]; j < 80 has no source -> zero
                    nc.vector.memset(t_x[0:BC, 0:SW], 0.0)
                    nc.gpsimd.dma_start(
                        out=t_x[0:BC, SW : SW + pass_len],
                        in_=bass.AP(sub, 0, [[CHW, BC], [1, pass_len]]),
                    )
                else:
                    nc.gpsimd.dma_start(
                        out=t_x[0:BC, 0:pass_len],
                        in_=bass.AP(sub, c0 - SW, [[CHW, BC], [1, pass_len]]),
                    )
                for m0 in range(0, pass_len, mm_f):
                    mf = min(mm_f, pass_len - m0)
                    t_p = ppool.tile([BC, mm_f], _F32, tag="psum")
                    nc.tensor.matmul(
                        t_p[:, 0:mf], t_w[:, :], t_x[:, m0 : m0 + mf]
                    )
                    nc.vector.tensor_copy(t_b[:, m0 : m0 + mf], t_p[:, 0:mf])
                nc.gpsimd.dma_start(
                    out=bass.AP(out, MID_OFF + c0, [[CHW, BC], [1, pass_len]]),
                    in_=t_b[:, :],
                )
    nc.compile()
    return nc


def _make_wsel():
    K = BC + C
    w = np.zeros((K, BC), dtype=np.float32)
    for bc in range(BC):
        w[bc, bc] = 1.0  # identity for the shifted sub rows (partitions 0..95)
        w[BC + bc % C, bc] = 1.0  # base channel selector (partitions 96..98)
    return w


def run(inputs, cfg=None, trace=False, **kw):
    sub = np.ascontiguousarray(inputs["subimg"], dtype=np.float32)
    basei = np.ascontiguousarray(inputs["base_image"], dtype=np.float32)
    assert sub.shape == (B, C, H, W) and basei.shape == (1, C, H, W)

    nc = build_nc(cfg)
    w = _make_wsel()
    in_maps = [
        {"subimg": sub[i * BS : (i + 1) * BS], "base": basei[0], "wsel": w}
        for i in range(N_CORES)
    ]
    res = run_bass_kernel_spmd(nc, in_maps, list(range(N_CORES)), trace=trace, **kw)
    full = np.concatenate(
        [res.results[i]["out"] for i in range(N_CORES)], axis=0
    ).astype(np.float32, copy=False)
    return full, res


def kernel(**inputs) -> np.ndarray:
    out, _ = run(inputs)
    return out


# revision 8
# speedup vs baseline: 5.0994x; 5.0994x over previous
"""Trainium2 kernel for ApplyStickerLayer: out = roll(subimg, (80,80), (2,3)) + base_image.

Structure (guaranteed by the layer): subimg is zero outside the 50x50 sticker
at the origin, base_image is zero inside the destination window, and the roll
never wraps -- so per (b, c) channel image (flat, 50176 elems):

    out[bc, f] = base[bc % 3, f] + sub[bc, f - 18000]     (sub oob -> 0)

HW findings driving this design (measured on this part):
  * SBUF AXI port coverage is king: partition p is wired to one of 16 ports.
    DMAs sourcing from partitions 0..15 get ~2 ports (~50 GB/s); partitions
    32..127 cover ALL 16 ports.  So every store sources from a [128, f] tile
    using rows 32..127.
  * SWDGE descriptors cost ~0.7 us fixed (HBM write round trip), so
    descriptors are fat: one 44.8 KB run per (bc) image column chunk.
  * Cross-partition broadcast is only cheap on TensorE: one matmul per column
    chunk replicates base into all 96 images and adds the shifted sticker:

        psum[128, f] = W.T @ x    W [99, 128] = [identity ; channel selector]
                                  x [99, f]   = [96 sub rows ; 3 base rows]

    (columns 0..31 of W are zero; psum rows 32..127 hold images 0..95).
    Pure-base chunks (f outside [18000, 29200)) use only the 3 selector rows.
  * Inputs are cast f32->bf16 during the load DMA; the matmul accumulates in
    f32.  bf16 rounding (~0.4% rel) is far inside the 2e-2 gate.

DVE drains PSUM to SBUF f32 tiles; SWDGE stores them as 96 fat descriptors
per chunk.  Per core ~19.3 MB written + ~4.9 MB read.
"""

import sys

import numpy as np

if "/opt/trn_rl_repo" not in sys.path:
    sys.path.insert(0, "/opt/trn_rl_repo")

import concourse.bacc as bacc
import concourse.bass as bass
import concourse.mybir as mybir
import concourse.tile as tile
from concourse.bass_utils import run_bass_kernel_spmd

N_CORES = 8
B, C, H, W = 256, 3, 224, 224
BS = B // N_CORES  # 32 batches per core
BC = BS * C  # 96 channel images per core
SH, SW = 80, 80
KH, KW = 50, 50

CHW = H * W  # 50176
IMG = C * CHW  # 150528
SHIFT = SH * W + SW  # 18000: the roll as a flat shift
SUB_LEN = (KH - 1) * W + KW + (W - KW)  # 11200: sub cols that can be nonzero
# shifted-sub support inside a channel image: [SHIFT, SHIFT + SUB_LEN)

K = BC + C  # 99: matmul contraction (96 sub rows + 3 base rows)
P0 = 32  # psum/store rows are 32..127 (covers all 16 SBUF AXI ports)

_F32 = mybir.dt.float32
_BF16 = mybir.dt.bfloat16

DEFAULT_CFG = {
    "fc": 11200,  # column chunk (also the store descriptor length / 4)
    "mm_f": 512,  # matmul free-dim chunk (<= 512, one PSUM bank)
    "psum_bufs": 4,
    "out_bufs": 2,
    "x_bufs": 2,
    "swq": 1,  # num_swdge_queues
}


def build_nc(cfg=None):
    cfg = {**DEFAULT_CFG, **(cfg or {})}
    fc_max = cfg["fc"]
    mm_f = cfg["mm_f"]

    nc = bacc.Bacc(
        "TRN2",
        target_bir_lowering=False,
        num_devices=N_CORES,
        num_swdge_queues=cfg["swq"],
    )
    sub = nc.declare_dram_parameter("subimg", [BS, C, H, W], _F32, isOutput=False)
    base = nc.declare_dram_parameter("base", [C, H, W], _F32, isOutput=False)
    wsel = nc.declare_dram_parameter("wsel", [K, 128], _F32, isOutput=False)
    out = nc.declare_dram_parameter("out", [BS, C, H, W], _F32, isOutput=True)

    chunks = []
    c0 = 0
    while c0 < CHW:
        chunks.append((c0, min(fc_max, CHW - c0)))
        c0 += fc_max

    with tile.TileContext(nc) as tc:
        with (
            tc.tile_pool(name="consts", bufs=1) as cpool,
            tc.tile_pool(name="work", bufs=1) as wpool,
            tc.tile_pool(name="psum", bufs=cfg["psum_bufs"], space=bass.MemorySpace.PSUM) as ppool,
        ):
            t_wk = cpool.tile([K, 128], _BF16, tag="wk")
            nc.gpsimd.dma_start(out=t_wk[:, :], in_=wsel[:, :])
            t_w3 = cpool.tile([C, 128], _BF16, tag="w3")
            nc.gpsimd.dma_start(out=t_w3[:, :], in_=wsel[BC:K, :])

            for c0, fc in chunks:
                # sub columns contributing to out cols [c0, c0+fc):
                # sub j = f - SHIFT clipped to [0, SUB_LEN)
                s_lo = max(0, c0 - SHIFT)
                s_hi = min(SUB_LEN, c0 + fc - SHIFT)
                has_sub = s_hi > s_lo

                if has_sub:
                    t_x = wpool.tile([K, fc_max], _BF16, tag="x", bufs=cfg["x_bufs"])
                    x_lo = s_lo + SHIFT - c0  # x column where sub j = s_lo lands
                    x_hi = x_lo + (s_hi - s_lo)
                    if x_lo > 0:
                        nc.vector.memset(t_x[0:BC, 0:x_lo], 0.0)
                    if x_hi < fc:
                        nc.vector.memset(t_x[0:BC, x_hi:fc], 0.0)
                    nc.gpsimd.dma_start(
                        out=t_x[0:BC, x_lo:x_hi],
                        in_=bass.AP(sub, s_lo, [[CHW, BC], [1, s_hi - s_lo]]),
                    )
                    nc.gpsimd.dma_start(
                        out=t_x[BC:K, 0:fc],
                        in_=bass.AP(base, c0, [[CHW, C], [1, fc]]),
                    )
                else:
                    t_x = wpool.tile([C, fc_max], _BF16, tag="xb", bufs=1)
                    nc.gpsimd.dma_start(
                        out=t_x[0:C, 0:fc],
                        in_=bass.AP(base, c0, [[CHW, C], [1, fc]]),
                    )

                t_o = wpool.tile([128, fc_max], _F32, tag="out", bufs=cfg["out_bufs"])
                for m0 in range(0, fc, mm_f):
                    mf = min(mm_f, fc - m0)
                    t_p = ppool.tile([128, mm_f], _F32, tag="psum")
                    if has_sub:
                        nc.tensor.matmul(
                            t_p[:, 0:mf], t_wk[:, :], t_x[:, m0 : m0 + mf]
                        )
                    else:
                        nc.tensor.matmul(
                            t_p[:, 0:mf], t_w3[:, :], t_x[0:C, m0 : m0 + mf]
                        )
                    # full 128 lanes (rows 0..31 are dead weight): DVE APs
                    # with a nonzero start partition are capped at 32 rows
                    nc.vector.tensor_copy(
                        t_o[:, m0 : m0 + mf], t_p[:, 0:mf]
                    )
                nc.gpsimd.dma_start(
                    out=bass.AP(out, c0, [[CHW, BC], [1, fc]]),
                    in_=t_o[P0:128, 0:fc],
                )
    nc.compile()
    return nc


def _make_wsel():
    w = np.zeros((K, 128), dtype=np.float32)
    for m in range(P0, 128):
        bc = m - P0
        w[bc, m] = 1.0  # identity for the shifted sub rows
        w[BC + bc % C, m] = 1.0  # base channel selector
    return w


def run(inputs, cfg=None, trace=False, **kw):
    sub = np.ascontiguousarray(inputs["subimg"], dtype=np.float32)
    basei = np.ascontiguousarray(inputs["base_image"], dtype=np.float32)
    assert sub.shape == (B, C, H, W) and basei.shape == (1, C, H, W)

    nc = build_nc(cfg)
    w = _make_wsel()
    in_maps = [
        {"subimg": sub[i * BS : (i + 1) * BS], "base": basei[0], "wsel": w}
        for i in range(N_CORES)
    ]
    res = run_bass_kernel_spmd(nc, in_maps, list(range(N_CORES)), trace=trace, **kw)
    full = np.concatenate(
        [res.results[i]["out"] for i in range(N_CORES)], axis=0
    ).astype(np.float32, copy=False)
    return full, res


def kernel(**inputs) -> np.ndarray:
    out, _ = run(inputs)
    return out


# revision 10
# speedup vs baseline: 5.6771x; 1.1133x over previous
"""Trainium2 kernel for ApplyStickerLayer: out = roll(subimg, (80,80), (2,3)) + base_image.

Structure (guaranteed by the layer): subimg is zero outside the 50x50 sticker
at the origin, base_image is zero inside the destination window, and the roll
never wraps -- so per (b, c) channel image (flat, 50176 elems):

    out[bc, f] = base[bc % 3, f] + sub[bc, f - 18000]     (sub oob -> 0)

HW findings driving this design (measured on this part):
  * SBUF AXI port coverage is king: partition p is wired to one of 16 ports.
    DMAs sourcing from partitions 0..15 get ~2 ports (~50 GB/s); partitions
    32..127 cover ALL 16 ports.  So every store sources from a [128, f] tile
    using rows 32..127.
  * SWDGE descriptors cost ~0.7 us fixed (HBM write round trip), so
    descriptors are fat: one 44.8 KB run per (bc) image column chunk.
  * Cross-partition broadcast is only cheap on TensorE: one matmul per column
    chunk replicates base into all 96 images and adds the shifted sticker:

        psum[128, f] = W.T @ x    W [99, 128] = [identity ; channel selector]
                                  x [99, f]   = [96 sub rows ; 3 base rows]

    (columns 0..31 of W are zero; psum rows 32..127 hold images 0..95).
    Pure-base chunks (f outside [18000, 29200)) use only the 3 selector rows.
  * Inputs are cast f32->bf16 during the load DMA; the matmul accumulates in
    f32.  bf16 rounding (~0.4% rel) is far inside the 2e-2 gate.

DVE drains PSUM to SBUF f32 tiles; SWDGE stores them as 96 fat descriptors
per chunk.  Per core ~19.3 MB written + ~4.9 MB read.
"""

import sys

import numpy as np

if "/opt/trn_rl_repo" not in sys.path:
    sys.path.insert(0, "/opt/trn_rl_repo")

import concourse.bacc as bacc
import concourse.bass as bass
import concourse.mybir as mybir
import concourse.tile as tile
from concourse.bass_utils import run_bass_kernel_spmd

N_CORES = 8
B, C, H, W = 256, 3, 224, 224
BS = B // N_CORES  # 32 batches per core
BC = BS * C  # 96 channel images per core
SH, SW = 80, 80
KH, KW = 50, 50

CHW = H * W  # 50176
IMG = C * CHW  # 150528
SHIFT = SH * W + SW  # 18000: the roll as a flat shift
SUB_LEN = (KH - 1) * W + KW + (W - KW)  # 11200: sub cols that can be nonzero
# shifted-sub support inside a channel image: [SHIFT, SHIFT + SUB_LEN)

K = BC + C  # 99: matmul contraction (96 sub rows + 3 base rows)

_F32 = mybir.dt.float32
_BF16 = mybir.dt.bfloat16

DEFAULT_CFG = {
    "fc": 5600,  # column chunk (also the store descriptor length / 4)
    "mm_f": 512,  # matmul free-dim chunk (<= 512, one PSUM bank)
    "psum_bufs": 6,
    "out_bufs": 3,
    "x_bufs": 2,
    "xb_bufs": 2,
    "act_every": 4,  # every act_every-th PSUM evac goes to ScalarE (ACT)
    "swq": 1,  # num_swdge_queues
}


def build_nc(cfg=None):
    cfg = {**DEFAULT_CFG, **(cfg or {})}
    fc_max = cfg["fc"]
    mm_f = cfg["mm_f"]

    nc = bacc.Bacc(
        "TRN2",
        target_bir_lowering=False,
        num_devices=N_CORES,
        num_swdge_queues=cfg["swq"],
    )
    sub = nc.declare_dram_parameter("subimg", [BS, C, H, W], _F32, isOutput=False)
    base = nc.declare_dram_parameter("base", [C, H, W], _F32, isOutput=False)
    wsel = nc.declare_dram_parameter("wsel", [K, BC], _F32, isOutput=False)
    out = nc.declare_dram_parameter("out", [BS, C, H, W], _F32, isOutput=True)

    chunks = []
    c0 = 0
    while c0 < CHW:
        chunks.append((c0, min(fc_max, CHW - c0)))
        c0 += fc_max

    with tile.TileContext(nc) as tc:
        with (
            tc.tile_pool(name="consts", bufs=1) as cpool,
            tc.tile_pool(name="work", bufs=1) as wpool,
            tc.tile_pool(name="psum", bufs=cfg["psum_bufs"], space=bass.MemorySpace.PSUM) as ppool,
        ):
            t_wk = cpool.tile([K, BC], _BF16, tag="wk")
            nc.gpsimd.dma_start(out=t_wk[:, :], in_=wsel[:, :])
            t_w3 = cpool.tile([C, BC], _BF16, tag="w3")
            nc.gpsimd.dma_start(out=t_w3[:, :], in_=wsel[BC:K, :])

            for c0, fc in chunks:
                # sub columns contributing to out cols [c0, c0+fc):
                # sub j = f - SHIFT clipped to [0, SUB_LEN)
                s_lo = max(0, c0 - SHIFT)
                s_hi = min(SUB_LEN, c0 + fc - SHIFT)
                has_sub = s_hi > s_lo

                if has_sub:
                    t_x = wpool.tile([K, fc_max], _BF16, tag="x", bufs=cfg["x_bufs"])
                    x_lo = s_lo + SHIFT - c0  # x column where sub j = s_lo lands
                    x_hi = x_lo + (s_hi - s_lo)
                    if x_lo > 0:
                        nc.vector.memset(t_x[0:BC, 0:x_lo], 0.0)
                    if x_hi < fc:
                        nc.vector.memset(t_x[0:BC, x_hi:fc], 0.0)
                    nc.gpsimd.dma_start(
                        out=t_x[0:BC, x_lo:x_hi],
                        in_=bass.AP(sub, s_lo, [[CHW, BC], [1, s_hi - s_lo]]),
                    )
                    nc.gpsimd.dma_start(
                        out=t_x[BC:K, 0:fc],
                        in_=bass.AP(base, c0, [[CHW, C], [1, fc]]),
                    )
                else:
                    t_x = wpool.tile([C, fc_max], _BF16, tag="xb", bufs=cfg["xb_bufs"])
                    nc.gpsimd.dma_start(
                        out=t_x[0:C, 0:fc],
                        in_=bass.AP(base, c0, [[CHW, C], [1, fc]]),
                    )

                t_o = wpool.tile([BC, fc_max], _F32, tag="out", bufs=cfg["out_bufs"])
                for mi, m0 in enumerate(range(0, fc, mm_f)):
                    mf = min(mm_f, fc - m0)
                    t_p = ppool.tile([BC, mm_f], _F32, tag="psum")
                    if has_sub:
                        nc.tensor.matmul(
                            t_p[:, 0:mf], t_wk[:, :], t_x[:, m0 : m0 + mf]
                        )
                    else:
                        nc.tensor.matmul(
                            t_p[:, 0:mf], t_w3[:, :], t_x[0:C, m0 : m0 + mf]
                        )
                    # PSUM drain: mostly DVE, a slice to the idle ACT engine
                    if (mi % cfg["act_every"]) == cfg["act_every"] - 1:
                        nc.scalar.copy(t_o[:, m0 : m0 + mf], t_p[:, 0:mf])
                    else:
                        nc.vector.tensor_copy(t_o[:, m0 : m0 + mf], t_p[:, 0:mf])
                nc.gpsimd.dma_start(
                    out=bass.AP(out, c0, [[CHW, BC], [1, fc]]),
                    in_=t_o[:, 0:fc],
                )
    nc.compile()
    return nc


def _make_wsel():
    w = np.zeros((K, BC), dtype=np.float32)
    for bc in range(BC):
        w[bc, bc] = 1.0  # identity for the shifted sub rows
        w[BC + bc % C, bc] = 1.0  # base channel selector
    return w


def run(inputs, cfg=None, trace=False, **kw):
    sub = np.ascontiguousarray(inputs["subimg"], dtype=np.float32)
    basei = np.ascontiguousarray(inputs["base_image"], dtype=np.float32)
    assert sub.shape == (B, C, H, W) and basei.shape == (1, C, H, W)

    nc = build_nc(cfg)
    w = _make_wsel()
    in_maps = [
        {"subimg": sub[i * BS : (i + 1) * BS], "base": basei[0], "wsel": w}
        for i in range(N_CORES)
    ]
    res = run_bass_kernel_spmd(nc, in_maps, list(range(N_CORES)), trace=trace, **kw)
    full = np.concatenate(
        [res.results[i]["out"] for i in range(N_CORES)], axis=0
    ).astype(np.float32, copy=False)
    return full, res


def kernel(**inputs) -> np.ndarray:
    out, _ = run(inputs)
    return out


# revision 11
# speedup vs baseline: 6.5029x; 1.1454x over previous
"""Trainium2 kernel for ApplyStickerLayer: out = roll(subimg, (80,80), (2,3)) + base_image.

Structure (guaranteed by the layer): subimg is zero outside the 50x50 sticker
at the origin, base_image is zero inside the destination window, and the roll
never wraps -- so per (b, c) channel image (flat, 50176 elems):

    out[bc, f] = base[bc % 3, f] + sub[bc, f - 18000]     (sub oob -> 0)

HW findings driving this design (measured on this part):
  * SBUF AXI port coverage is king: partition p is wired to one of 16 ports.
    DMAs sourcing from partitions 0..15 get ~2 ports (~50 GB/s); partitions
    32..127 cover ALL 16 ports.  So every store sources from a [128, f] tile
    using rows 32..127.
  * SWDGE descriptors cost ~0.7 us fixed (HBM write round trip), so
    descriptors are fat: one 44.8 KB run per (bc) image column chunk.
  * Cross-partition broadcast is only cheap on TensorE: one matmul per column
    chunk replicates base into all 96 images and adds the shifted sticker:

        psum[128, f] = W.T @ x    W [99, 128] = [identity ; channel selector]
                                  x [99, f]   = [96 sub rows ; 3 base rows]

    (columns 0..31 of W are zero; psum rows 32..127 hold images 0..95).
    Pure-base chunks (f outside [18000, 29200)) use only the 3 selector rows.
  * Inputs are cast f32->bf16 during the load DMA; the matmul accumulates in
    f32.  bf16 rounding (~0.4% rel) is far inside the 2e-2 gate.

DVE drains PSUM to SBUF f32 tiles; SWDGE stores them as 96 fat descriptors
per chunk.  Per core ~19.3 MB written + ~4.9 MB read.
"""

import sys

import numpy as np

if "/opt/trn_rl_repo" not in sys.path:
    sys.path.insert(0, "/opt/trn_rl_repo")

import concourse.bacc as bacc
import concourse.bass as bass
import concourse.mybir as mybir
import concourse.tile as tile
from concourse.bass_utils import run_bass_kernel_spmd

N_CORES = 8
B, C, H, W = 256, 3, 224, 224
BS = B // N_CORES  # 32 batches per core
BC = BS * C  # 96 channel images per core
SH, SW = 80, 80
KH, KW = 50, 50

CHW = H * W  # 50176
IMG = C * CHW  # 150528
SHIFT = SH * W + SW  # 18000: the roll as a flat shift
SUB_LEN = (KH - 1) * W + KW + (W - KW)  # 11200: sub cols that can be nonzero
# shifted-sub support inside a channel image: [SHIFT, SHIFT + SUB_LEN)

K = BC + C  # 99: matmul contraction (96 sub rows + 3 base rows)

_F32 = mybir.dt.float32
_BF16 = mybir.dt.bfloat16

DEFAULT_CFG = {
    "fc": 5600,  # column chunk (also the store descriptor length / 4)
    "mm_f": 512,  # matmul free-dim chunk (<= 512, one PSUM bank)
    "psum_bufs": 8,
    "out_bufs": 3,
    "x_bufs": 3,
    "xb_bufs": 3,
    "act_every": 3,  # every act_every-th PSUM evac goes to ScalarE (ACT)
    "store_eng": "sync",  # HWDGE ring for stores; loads stay on SWDGE
    "swq": 1,  # num_swdge_queues
}


def build_nc(cfg=None):
    cfg = {**DEFAULT_CFG, **(cfg or {})}
    fc_max = cfg["fc"]
    mm_f = cfg["mm_f"]

    nc = bacc.Bacc(
        "TRN2",
        target_bir_lowering=False,
        num_devices=N_CORES,
        num_swdge_queues=cfg["swq"],
    )
    sub = nc.declare_dram_parameter("subimg", [BS, C, H, W], _F32, isOutput=False)
    base = nc.declare_dram_parameter("base", [C, H, W], _F32, isOutput=False)
    wsel = nc.declare_dram_parameter("wsel", [K, BC], _F32, isOutput=False)
    out = nc.declare_dram_parameter("out", [BS, C, H, W], _F32, isOutput=True)

    chunks = []
    c0 = 0
    while c0 < CHW:
        chunks.append((c0, min(fc_max, CHW - c0)))
        c0 += fc_max

    with tile.TileContext(nc) as tc:
        with (
            tc.tile_pool(name="consts", bufs=1) as cpool,
            tc.tile_pool(name="work", bufs=1) as wpool,
            tc.tile_pool(name="psum", bufs=cfg["psum_bufs"], space=bass.MemorySpace.PSUM) as ppool,
        ):
            t_wk = cpool.tile([K, BC], _BF16, tag="wk")
            nc.gpsimd.dma_start(out=t_wk[:, :], in_=wsel[:, :])
            t_w3 = cpool.tile([C, BC], _BF16, tag="w3")
            nc.gpsimd.dma_start(out=t_w3[:, :], in_=wsel[BC:K, :])

            for c0, fc in chunks:
                # sub columns contributing to out cols [c0, c0+fc):
                # sub j = f - SHIFT clipped to [0, SUB_LEN)
                s_lo = max(0, c0 - SHIFT)
                s_hi = min(SUB_LEN, c0 + fc - SHIFT)
                has_sub = s_hi > s_lo

                if has_sub:
                    t_x = wpool.tile([K, fc_max], _BF16, tag="x", bufs=cfg["x_bufs"])
                    x_lo = s_lo + SHIFT - c0  # x column where sub j = s_lo lands
                    x_hi = x_lo + (s_hi - s_lo)
                    if x_lo > 0:
                        nc.vector.memset(t_x[0:BC, 0:x_lo], 0.0)
                    if x_hi < fc:
                        nc.vector.memset(t_x[0:BC, x_hi:fc], 0.0)
                    nc.gpsimd.dma_start(
                        out=t_x[0:BC, x_lo:x_hi],
                        in_=bass.AP(sub, s_lo, [[CHW, BC], [1, s_hi - s_lo]]),
                    )
                    nc.gpsimd.dma_start(
                        out=t_x[BC:K, 0:fc],
                        in_=bass.AP(base, c0, [[CHW, C], [1, fc]]),
                    )
                else:
                    t_x = wpool.tile([C, fc_max], _BF16, tag="xb", bufs=cfg["xb_bufs"])
                    nc.gpsimd.dma_start(
                        out=t_x[0:C, 0:fc],
                        in_=bass.AP(base, c0, [[CHW, C], [1, fc]]),
                    )

                t_o = wpool.tile([BC, fc_max], _F32, tag="out", bufs=cfg["out_bufs"])
                for mi, m0 in enumerate(range(0, fc, mm_f)):
                    mf = min(mm_f, fc - m0)
                    t_p = ppool.tile([BC, mm_f], _F32, tag="psum")
                    if has_sub:
                        nc.tensor.matmul(
                            t_p[:, 0:mf], t_wk[:, :], t_x[:, m0 : m0 + mf]
                        )
                    else:
                        nc.tensor.matmul(
                            t_p[:, 0:mf], t_w3[:, :], t_x[0:C, m0 : m0 + mf]
                        )
                    # PSUM drain: mostly DVE, a slice to the idle ACT engine
                    if (mi % cfg["act_every"]) == cfg["act_every"] - 1:
                        nc.scalar.copy(t_o[:, m0 : m0 + mf], t_p[:, 0:mf])
                    else:
                        nc.vector.tensor_copy(t_o[:, m0 : m0 + mf], t_p[:, 0:mf])
                store_eng = getattr(nc, cfg["store_eng"])
                store_eng.dma_start(
                    out=bass.AP(out, c0, [[CHW, BC], [1, fc]]),
                    in_=t_o[:, 0:fc],
                )
    nc.compile()
    return nc


def _make_wsel():
    w = np.zeros((K, BC), dtype=np.float32)
    for bc in range(BC):
        w[bc, bc] = 1.0  # identity for the shifted sub rows
        w[BC + bc % C, bc] = 1.0  # base channel selector
    return w


def run(inputs, cfg=None, trace=False, **kw):
    sub = np.ascontiguousarray(inputs["subimg"], dtype=np.float32)
    basei = np.ascontiguousarray(inputs["base_image"], dtype=np.float32)
    assert sub.shape == (B, C, H, W) and basei.shape == (1, C, H, W)

    nc = build_nc(cfg)
    w = _make_wsel()
    in_maps = [
        {"subimg": sub[i * BS : (i + 1) * BS], "base": basei[0], "wsel": w}
        for i in range(N_CORES)
    ]
    res = run_bass_kernel_spmd(nc, in_maps, list(range(N_CORES)), trace=trace, **kw)
    full = np.concatenate(
        [res.results[i]["out"] for i in range(N_CORES)], axis=0
    ).astype(np.float32, copy=False)
    return full, res


def kernel(**inputs) -> np.ndarray:
    out, _ = run(inputs)
    return out


# revision 12
# speedup vs baseline: 6.5065x; 1.0006x over previous
"""Trainium2 kernel for ApplyStickerLayer: out = roll(subimg, (80,80), (2,3)) + base_image.

Structure (guaranteed by the layer): subimg is zero outside the 50x50 sticker
at the origin, base_image is zero inside the destination window, and the roll
never wraps -- so per (b, c) channel image (flat, 50176 elems):

    out[bc, f] = base[bc % 3, f] + sub[bc, f - 18000]     (sub oob -> 0)

HW findings driving this design (measured on this part):
  * SBUF AXI port coverage is king: partition p is wired to one of 16 ports.
    DMAs sourcing from partitions 0..15 get ~2 ports (~50 GB/s); partitions
    32..127 cover ALL 16 ports.  So every store sources from a [128, f] tile
    using rows 32..127.
  * SWDGE descriptors cost ~0.7 us fixed (HBM write round trip), so
    descriptors are fat: one 44.8 KB run per (bc) image column chunk.
  * Cross-partition broadcast is only cheap on TensorE: one matmul per column
    chunk replicates base into all 96 images and adds the shifted sticker:

        psum[128, f] = W.T @ x    W [99, 128] = [identity ; channel selector]
                                  x [99, f]   = [96 sub rows ; 3 base rows]

    (columns 0..31 of W are zero; psum rows 32..127 hold images 0..95).
    Pure-base chunks (f outside [18000, 29200)) use only the 3 selector rows.
  * Inputs are cast f32->bf16 during the load DMA; the matmul accumulates in
    f32.  bf16 rounding (~0.4% rel) is far inside the 2e-2 gate.

DVE drains PSUM to SBUF f32 tiles; SWDGE stores them as 96 fat descriptors
per chunk.  Per core ~19.3 MB written + ~4.9 MB read.
"""

import sys

import numpy as np

if "/opt/trn_rl_repo" not in sys.path:
    sys.path.insert(0, "/opt/trn_rl_repo")

import concourse.bacc as bacc
import concourse.bass as bass
import concourse.mybir as mybir
import concourse.tile as tile
from concourse.bass_utils import run_bass_kernel_spmd

N_CORES = 8
B, C, H, W = 256, 3, 224, 224
BS = B // N_CORES  # 32 batches per core
BC = BS * C  # 96 channel images per core
SH, SW = 80, 80
KH, KW = 50, 50

CHW = H * W  # 50176
IMG = C * CHW  # 150528
SHIFT = SH * W + SW  # 18000: the roll as a flat shift
SUB_LEN = (KH - 1) * W + KW + (W - KW)  # 11200: sub cols that can be nonzero
# shifted-sub support inside a channel image: [SHIFT, SHIFT + SUB_LEN)

K = BC + C  # 99: matmul contraction (96 sub rows + 3 base rows)

_F32 = mybir.dt.float32
_BF16 = mybir.dt.bfloat16

DEFAULT_CFG = {
    "fc": 5600,  # column chunk (also the store descriptor length / 4)
    "mm_f": 512,  # matmul free-dim chunk (<= 512, one PSUM bank)
    "psum_bufs": 8,
    "out_bufs": 4,
    "x_bufs": 4,
    "xb_bufs": 4,
    "act_every": 2,  # every act_every-th PSUM evac goes to ScalarE (ACT)
    "store_eng": "sync",  # HWDGE ring for stores; loads stay on SWDGE
    "swq": 1,  # num_swdge_queues
}


def build_nc(cfg=None):
    cfg = {**DEFAULT_CFG, **(cfg or {})}
    fc_max = cfg["fc"]
    mm_f = cfg["mm_f"]

    nc = bacc.Bacc(
        "TRN2",
        target_bir_lowering=False,
        num_devices=N_CORES,
        num_swdge_queues=cfg["swq"],
    )
    sub = nc.declare_dram_parameter("subimg", [BS, C, H, W], _F32, isOutput=False)
    base = nc.declare_dram_parameter("base", [C, H, W], _F32, isOutput=False)
    wsel = nc.declare_dram_parameter("wsel", [K, 128], _F32, isOutput=False)
    out = nc.declare_dram_parameter("out", [BS, C, H, W], _F32, isOutput=True)

    chunks = []
    c0 = 0
    while c0 < CHW:
        chunks.append((c0, min(fc_max, CHW - c0)))
        c0 += fc_max

    with tile.TileContext(nc) as tc:
        with (
            tc.tile_pool(name="consts", bufs=1) as cpool,
            tc.tile_pool(name="work", bufs=1) as wpool,
            tc.tile_pool(name="psum", bufs=cfg["psum_bufs"], space=bass.MemorySpace.PSUM) as ppool,
        ):
            # 128-wide weights: full-width LDWEIGHTS is ~2x faster than 96
            t_wk = cpool.tile([K, 128], _BF16, tag="wk")
            nc.gpsimd.dma_start(out=t_wk[:, :], in_=wsel[:, :])
            t_w3 = cpool.tile([C, 128], _BF16, tag="w3")
            nc.gpsimd.dma_start(out=t_w3[:, :], in_=wsel[BC:K, :])

            for c0, fc in chunks:
                # sub columns contributing to out cols [c0, c0+fc):
                # sub j = f - SHIFT clipped to [0, SUB_LEN)
                s_lo = max(0, c0 - SHIFT)
                s_hi = min(SUB_LEN, c0 + fc - SHIFT)
                has_sub = s_hi > s_lo

                if has_sub:
                    t_x = wpool.tile([K, fc_max], _BF16, tag="x", bufs=cfg["x_bufs"])
                    x_lo = s_lo + SHIFT - c0  # x column where sub j = s_lo lands
                    x_hi = x_lo + (s_hi - s_lo)
                    if x_lo > 0:
                        nc.vector.memset(t_x[0:BC, 0:x_lo], 0.0)
                    if x_hi < fc:
                        nc.vector.memset(t_x[0:BC, x_hi:fc], 0.0)
                    nc.gpsimd.dma_start(
                        out=t_x[0:BC, x_lo:x_hi],
                        in_=bass.AP(sub, s_lo, [[CHW, BC], [1, s_hi - s_lo]]),
                    )
                    nc.gpsimd.dma_start(
                        out=t_x[BC:K, 0:fc],
                        in_=bass.AP(base, c0, [[CHW, C], [1, fc]]),
                    )
                else:
                    t_x = wpool.tile([C, fc_max], _BF16, tag="xb", bufs=cfg["xb_bufs"])
                    nc.gpsimd.dma_start(
                        out=t_x[0:C, 0:fc],
                        in_=bass.AP(base, c0, [[CHW, C], [1, fc]]),
                    )

                t_o = wpool.tile([BC, fc_max], _F32, tag="out", bufs=cfg["out_bufs"])
                for mi, m0 in enumerate(range(0, fc, mm_f)):
                    mf = min(mm_f, fc - m0)
                    t_p = ppool.tile([128, mm_f], _F32, tag="psum")
                    if has_sub:
                        nc.tensor.matmul(
                            t_p[:, 0:mf], t_wk[:, :], t_x[:, m0 : m0 + mf]
                        )
                    else:
                        nc.tensor.matmul(
                            t_p[:, 0:mf], t_w3[:, :], t_x[0:C, m0 : m0 + mf]
                        )
                    # PSUM drain: mostly DVE, a slice to the idle ACT engine
                    if (mi % cfg["act_every"]) == cfg["act_every"] - 1:
                        nc.scalar.copy(t_o[:, m0 : m0 + mf], t_p[0:BC, 0:mf])
                    else:
                        nc.vector.tensor_copy(t_o[:, m0 : m0 + mf], t_p[0:BC, 0:mf])
                store_eng = getattr(nc, cfg["store_eng"])
                store_eng.dma_start(
                    out=bass.AP(out, c0, [[CHW, BC], [1, fc]]),
                    in_=t_o[:, 0:fc],
                )
    nc.compile()
    return nc


def _make_wsel():
    w = np.zeros((K, 128), dtype=np.float32)
    for bc in range(BC):
        w[bc, bc] = 1.0  # identity for the shifted sub rows
        w[BC + bc % C, bc] = 1.0  # base channel selector
    return w


def run(inputs, cfg=None, trace=False, **kw):
    sub = np.ascontiguousarray(inputs["subimg"], dtype=np.float32)
    basei = np.ascontiguousarray(inputs["base_image"], dtype=np.float32)
    assert sub.shape == (B, C, H, W) and basei.shape == (1, C, H, W)

    nc = build_nc(cfg)
    w = _make_wsel()
    in_maps = [
        {"subimg": sub[i * BS : (i + 1) * BS], "base": basei[0], "wsel": w}
        for i in range(N_CORES)
    ]
    res = run_bass_kernel_spmd(nc, in_maps, list(range(N_CORES)), trace=trace, **kw)
    full = np.concatenate(
        [res.results[i]["out"] for i in range(N_CORES)], axis=0
    ).astype(np.float32, copy=False)
    return full, res


def kernel(**inputs) -> np.ndarray:
    out, _ = run(inputs)
    return out
